# revision 4
# baseline (speedup 1.0000x reference)
"""Trainium2 Bass kernel for NaiveFourierKANLayer.

y[b,j] = sum_{i,g} cos(x[b,i]*k_g) * W[0,j,i,g] + sin(x[b,i]*k_g) * W[1,j,i,g]

B=4096, I=128, O=512, G=300.  Equivalent to a (B x K) @ (K x O) matmul with
K = 2*I*G = 76800 where the lhs rows are cos/sin of x*k, generated on-chip.

Sharding: the (g, d) contraction is split across the 8 cores (G padded to
304 -> 38 g's per core, both cos+sin terms).  Each core computes a full
[4096, 512] partial product; the host sums the 8 partials.  This keeps every
core's matmul shape identical (one compiled SPMD program) and cuts the
coefficient traffic per core 8x vs batch-data-parallel.

Per core, per b-group of 1024 (4 passes):
  for g in 38:   a = xT * (k_g/2pi)          (DVE, per-partition scalar)
                 n = (a + 1.5*2^23) - same   (DVE round-to-nearest)
                 f = a - n in [-0.5, 0.5]    (DVE)
                 fc = wrap(f + 0.25)         (DVE custom op, cos argument)
                 sin = Sin(2pi*f)  cos = Sin(2pi*fc)   (ACT, fp32r out)
                 16 fp32r matmuls [K=128 i] x [M=128 b] x [N=512 j]
                 accumulating into 8 PSUM banks (b-chunks of 128)
"""
import numpy as np

B, I, O, G = 4096, 128, 512, 300
NCORES = 8
GPAD = 304                  # 8 * 38
G_LOC = GPAD // NCORES      # 38 g's per core
BGRP = 1024                 # b-group per pass (8 psum banks x 128)
NPASS = B // BGRP           # 4
NCHUNK = BGRP // 128        # 8

MAGIC = float(np.float32(1.5 * 2 ** 23))
S2PI = float(np.float32(6.2831845))   # slightly < 2*pi so |f|*S2PI <= pi

_compiled = None


def _build():
    import concourse.bass as bass  # noqa: F401
    import concourse.mybir as mybir
    import concourse.tile as tile
    from concourse import bacc
    from concourse.alu_op_type import AluOpType

    f32 = mybir.dt.float32
    f32r = mybir.dt.float32r
    Sin = mybir.ActivationFunctionType.Sin

    nc = bacc.Bacc("TRN2", target_bir_lowering=False, debug=False,
                   num_devices=NCORES)
    xt_d = nc.dram_tensor("xt", [I, B], f32, kind="ExternalInput").ap()
    w_d = nc.dram_tensor("w", [G_LOC, 2, I, O], f32r, kind="ExternalInput").ap()
    sv_d = nc.dram_tensor("sv", [I, G_LOC], f32, kind="ExternalInput").ap()
    y_d = nc.dram_tensor("yp", [B, O], f32, kind="ExternalOutput").ap()

    with tile.TileContext(nc) as tc:
        with (
            tc.tile_pool(name="inp", bufs=1) as inp,
            tc.tile_pool(name="wpool", bufs=6) as wpool,
            tc.tile_pool(name="trig", bufs=3) as trig,
            tc.tile_pool(name="psum", bufs=1, space="PSUM") as pp,
            tc.tile_pool(name="opool", bufs=4) as opool,
        ):
            xt = inp.tile([I, B], f32)
            nc.sync.dma_start(xt[:], xt_d)
            sv = inp.tile([I, G_LOC], f32)
            nc.sync.dma_start(sv[:], sv_d)

            for p in range(NPASS):
                ps = [pp.tile([128, O], f32, tag=f"ps{c}", name=f"ps{c}")
                      for c in range(NCHUNK)]
                xs = xt[:, p * BGRP:(p + 1) * BGRP]
                for g in range(G_LOC):
                    wc = wpool.tile([I, O], f32r, tag="wc", name="wc")
                    nc.sync.dma_start(wc[:], w_d[g, 0])
                    ws = wpool.tile([I, O], f32r, tag="ws", name="ws")
                    nc.sync.dma_start(ws[:], w_d[g, 1])

                    a = trig.tile([I, BGRP], f32, tag="a", name="a")
                    n = trig.tile([I, BGRP], f32, tag="n", name="n")
                    f = trig.tile([I, BGRP], f32, tag="f", name="f")
                    fc = trig.tile([I, BGRP], f32, tag="fc", name="fc")
                    sn = trig.tile([I, BGRP], f32r, tag="sn", name="sn")
                    cs = trig.tile([I, BGRP], f32r, tag="cs", name="cs")
                    nc.vector.tensor_scalar(a[:], xs, sv[:, g:g + 1], None,
                                            AluOpType.mult)
                    nc.vector.tensor_scalar(n[:], a[:], MAGIC, MAGIC,
                                            AluOpType.add, AluOpType.subtract)
                    nc.vector.tensor_tensor(f[:], a[:], n[:],
                                            AluOpType.subtract)
                    nc.vector.add_range_wrap(fc[:], f[:], 0.25, 0.5, 1.0)
                    nc.scalar.activation(sn[:], f[:], Sin, scale=S2PI)
                    nc.scalar.activation(cs[:], fc[:], Sin, scale=S2PI)
                    for c in range(NCHUNK):
                        nc.tensor.matmul(ps[c][:],
                                         cs[:, c * 128:(c + 1) * 128],
                                         wc[:], start=(g == 0), stop=False)
                    for c in range(NCHUNK):
                        nc.tensor.matmul(ps[c][:],
                                         sn[:, c * 128:(c + 1) * 128],
                                         ws[:], start=False,
                                         stop=(g == G_LOC - 1))
                for c in range(NCHUNK):
                    o = opool.tile([128, O], f32, tag="o", name="o")
                    nc.scalar.copy(o[:], ps[c][:])
                    nc.sync.dma_start(y_d[p * BGRP + c * 128:
                                          p * BGRP + (c + 1) * 128, :], o[:])

    nc.compile()
    return nc


def _prep(x, fouriercoeffs):
    xt = np.ascontiguousarray(x.T.astype(np.float32, copy=False))  # [I, B]
    wp = np.zeros((GPAD, 2, I, O), dtype=np.float32)
    # fouriercoeffs[d, j, i, g] -> wp[g, d, i, j]
    wp[:G] = fouriercoeffs.transpose(3, 0, 2, 1)
    ks = np.arange(1, GPAD + 1, dtype=np.float64) / (2 * np.pi)
    sva = ks.astype(np.float32)
    sva[G:] = 0.0
    in_maps = []
    for m in range(NCORES):
        sl = slice(m * G_LOC, (m + 1) * G_LOC)
        in_maps.append({
            "xt": xt,
            "w": np.ascontiguousarray(wp[sl]),
            "sv": np.broadcast_to(sva[sl], (I, G_LOC)).copy(),
        })
    return in_maps


def kernel(x, fouriercoeffs):
    global _compiled
    from concourse.bass_utils import run_bass_kernel_spmd

    if _compiled is None:
        _compiled = _build()
    in_maps = _prep(np.asarray(x), np.asarray(fouriercoeffs))
    res = run_bass_kernel_spmd(_compiled, in_maps, core_ids=list(range(NCORES)))
    y = np.zeros((B, O), dtype=np.float64)
    for m in range(NCORES):
        y += res.results[m]["yp"].astype(np.float64)
    return y.astype(np.float32)


# revision 11
# speedup vs baseline: 1.1946x; 1.1946x over previous
"""Trainium2 Bass kernel for NaiveFourierKANLayer.

y[b,j] = sum_{i,g} cos(x[b,i]*k_g) * W[0,j,i,g] + sin(x[b,i]*k_g) * W[1,j,i,g]

B=4096, I=128, O=512, G=300.  Equivalent to a (B x K) @ (K x O) matmul with
K = 2*I*G = 76800 where the lhs rows are cos/sin of x*k, generated on-chip.

Sharding: the (g, d) contraction is split across the 8 cores (G padded to
304 -> 38 g's per core, both cos+sin terms).  Each core computes a full
[4096, 512] partial product; the host sums the 8 partials.  This keeps every
core's matmul shape identical (one compiled SPMD program) and cuts the
coefficient traffic per core 8x vs batch-data-parallel.

Per core, per b-group of 1024 (4 passes):
  for g in 38:   a = xT * (k_g/2pi)          (DVE, per-partition scalar)
                 n = (a + 1.5*2^23) - same   (DVE round-to-nearest)
                 f = a - n in [-0.5, 0.5]    (DVE)
                 fc = wrap(f + 0.25)         (DVE custom op, cos argument)
                 sin = Sin(2pi*f)  cos = Sin(2pi*fc)   (ACT, fp32r out)
                 16 fp32r matmuls [K=128 i] x [M=128 b] x [N=512 j]
                 accumulating into 8 PSUM banks (b-chunks of 128)
"""
import numpy as np

B, I, O, G = 4096, 128, 512, 300
NCORES = 8
GPAD = 304                  # 8 * 38
G_LOC = GPAD // NCORES      # 38 g's per core
BGRP = 1024                 # b-group per pass (8 psum banks x 128)
NPASS = B // BGRP           # 4
NCHUNK = BGRP // 128        # 8

MAGIC = float(np.float32(1.5 * 2 ** 23))
S2PI = float(np.float32(6.2831845))   # slightly < 2*pi so |f|*S2PI <= pi

_compiled = None


def _build():
    import concourse.bass as bass  # noqa: F401
    import concourse.mybir as mybir
    import concourse.tile as tile
    from concourse import bacc
    from concourse.alu_op_type import AluOpType

    f32 = mybir.dt.float32
    f32r = mybir.dt.float32r
    Sin = mybir.ActivationFunctionType.Sin
    Abs = mybir.ActivationFunctionType.Abs

    nc = bacc.Bacc("TRN2", target_bir_lowering=False, debug=False,
                   num_devices=NCORES)
    xt_d = nc.dram_tensor("xt", [I, B], f32, kind="ExternalInput").ap()
    w_d = nc.dram_tensor("w", [G_LOC, 2, I, O], f32r, kind="ExternalInput").ap()
    sv_d = nc.dram_tensor("sv", [I, G_LOC], f32, kind="ExternalInput").ap()
    y_d = nc.dram_tensor("yp", [B, O], f32, kind="ExternalOutput").ap()

    with tile.TileContext(nc) as tc:
        with (
            tc.tile_pool(name="inp", bufs=1) as inp,
            tc.tile_pool(name="wpool", bufs=6) as wpool,
            tc.tile_pool(name="trig", bufs=3) as trig,
            tc.tile_pool(name="psum", bufs=1, space="PSUM") as pp,
            tc.tile_pool(name="opool", bufs=4) as opool,
        ):
            xt = inp.tile([I, B], f32)
            nc.sync.dma_start(xt[:], xt_d)
            sv = inp.tile([I, G_LOC], f32)
            nc.sync.dma_start(sv[:], sv_d)
            bias_ph = inp.tile([I, 1], f32)
            nc.vector.memset(bias_ph[:], float(np.float32(np.pi / 2)))

            for p in range(NPASS):
                ps = [pp.tile([128, O], f32, tag=f"ps{c}", name=f"ps{c}")
                      for c in range(NCHUNK)]
                xs = xt[:, p * BGRP:(p + 1) * BGRP]
                for g in range(G_LOC):
                    wc = wpool.tile([I, O], f32r, tag="wc", name="wc")
                    nc.sync.dma_start(wc[:], w_d[g, 0])
                    ws = wpool.tile([I, O], f32r, tag="ws", name="ws")
                    nc.sync.dma_start(ws[:], w_d[g, 1])

                    a = trig.tile([I, BGRP], f32, tag="a", name="a")
                    n = trig.tile([I, BGRP], f32, tag="n", name="n")
                    f = trig.tile([I, BGRP], f32, tag="f", name="f")
                    fc = trig.tile([I, BGRP], f32, tag="fc", name="fc")
                    sn = trig.tile([I, BGRP], f32r, tag="sn", name="sn")
                    cs = trig.tile([I, BGRP], f32r, tag="cs", name="cs")
                    nc.vector.tensor_scalar(a[:], xs, sv[:, g:g + 1], None,
                                            AluOpType.mult)
                    nc.vector.tensor_scalar(n[:], a[:], MAGIC, MAGIC,
                                            AluOpType.add, AluOpType.subtract)
                    nc.vector.tensor_tensor(f[:], a[:], n[:],
                                            AluOpType.subtract)
                    nc.scalar.activation(sn[:], f[:], Sin, scale=S2PI)
                    if g % 2 == 0:
                        # cos arg on DVE: fc = wrap(f + 0.25) in turns
                        nc.vector.add_range_wrap(fc[:], f[:], 0.25, 0.5, 1.0)
                        nc.scalar.activation(cs[:], fc[:], Sin, scale=S2PI)
                    else:
                        # cos arg on ACT: |f|, then cos = Sin(pi/2 - 2pi|f|)
                        nc.scalar.activation(fc[:], f[:], Abs)
                        nc.scalar.activation(cs[:], fc[:], Sin, scale=-S2PI,
                                             bias=bias_ph[:, 0:1])
                    for c in range(NCHUNK):
                        nc.tensor.matmul(ps[c][:],
                                         cs[:, c * 128:(c + 1) * 128],
                                         wc[:], start=(g == 0), stop=False)
                    for c in range(NCHUNK):
                        nc.tensor.matmul(ps[c][:],
                                         sn[:, c * 128:(c + 1) * 128],
                                         ws[:], start=False,
                                         stop=(g == G_LOC - 1))
                for c in range(NCHUNK):
                    o = opool.tile([128, O], f32, tag="o", name="o")
                    nc.vector.tensor_copy(o[:], ps[c][:])
                    nc.sync.dma_start(y_d[p * BGRP + c * 128:
                                          p * BGRP + (c + 1) * 128, :], o[:])

    nc.compile()
    return nc


def _prep(x, fouriercoeffs):
    xt = np.ascontiguousarray(x.T.astype(np.float32, copy=False))  # [I, B]
    wp = np.zeros((GPAD, 2, I, O), dtype=np.float32)
    # fouriercoeffs[d, j, i, g] -> wp[g, d, i, j]
    wp[:G] = fouriercoeffs.transpose(3, 0, 2, 1)
    ks = np.arange(1, GPAD + 1, dtype=np.float64) / (2 * np.pi)
    sva = ks.astype(np.float32)
    sva[G:] = 0.0
    in_maps = []
    for m in range(NCORES):
        sl = slice(m * G_LOC, (m + 1) * G_LOC)
        in_maps.append({
            "xt": xt,
            "w": np.ascontiguousarray(wp[sl]),
            "sv": np.broadcast_to(sva[sl], (I, G_LOC)).copy(),
        })
    return in_maps


def kernel(x, fouriercoeffs):
    global _compiled
    from concourse.bass_utils import run_bass_kernel_spmd

    if _compiled is None:
        _compiled = _build()
    in_maps = _prep(np.asarray(x), np.asarray(fouriercoeffs))
    res = run_bass_kernel_spmd(_compiled, in_maps, core_ids=list(range(NCORES)))
    y = np.zeros((B, O), dtype=np.float64)
    for m in range(NCORES):
        y += res.results[m]["yp"].astype(np.float64)
    return y.astype(np.float32)


# revision 25
# speedup vs baseline: 1.2052x; 1.0089x over previous
"""Trainium2 Bass kernel for NaiveFourierKANLayer.

y[b,j] = sum_{i,g} cos(x[b,i]*k_g) * W[0,j,i,g] + sin(x[b,i]*k_g) * W[1,j,i,g]

B=4096, I=128, O=512, G=300.  Equivalent to a (B x K) @ (K x O) matmul with
K = 2*I*G = 76800 where the lhs rows are cos/sin of x*k, generated on-chip.

Sharding: the (g, d) contraction is split across the 8 cores (G padded to
304 -> 38 g's per core, both cos+sin terms).  Each core computes a full
[4096, 512] partial product; the host sums the 8 partials.  This keeps every
core's matmul shape identical (one compiled SPMD program) and cuts the
coefficient traffic per core 8x vs batch-data-parallel.

Per core, per b-group of 1024 (4 passes):
  for g in 38:   a = xT * (k_g/2pi)          (DVE, per-partition scalar)
                 n = (a + 1.5*2^23) - same   (DVE round-to-nearest)
                 f = a - n in [-0.5, 0.5]    (DVE)
                 cos argument: alternate by g parity between
                   DVE add_range_wrap(f+0.25) and ACT Abs + negated Sin
                   affine, to balance DVE/ACT load under the PE
                 sin = Sin(2pi*f)  cos = Sin(+-2pi*fc [+pi/2])  (ACT, f32r)
                 16 fp32r matmuls [K=128 i] x [M=128 b] x [N=512 j]
                 accumulating into 8 PSUM banks (b-chunks of 128)

Measured: ~583 us HW exec (8 cores), rel err ~1.2e-4 vs the fp32 reference.
PE runs gap-free at ~227.5 ns per [128x128x512] fp32r matmul (~93.5% of the
trace span; the rest is a ~20 us head and ~15 us drain+barrier tail).
"""
import numpy as np

B, I, O, G = 4096, 128, 512, 300
NCORES = 8
GPAD = 304                  # 8 * 38
G_LOC = GPAD // NCORES      # 38 g's per core
BGRP = 1024                 # b-group per pass (8 psum banks x 128)
NPASS = B // BGRP           # 4
NCHUNK = BGRP // 128        # 8

MAGIC = float(np.float32(1.5 * 2 ** 23))
S2PI = float(np.float32(6.2831845))   # slightly < 2*pi so |f|*S2PI <= pi

_compiled = None


def _build():
    import concourse.bass as bass  # noqa: F401
    import concourse.mybir as mybir
    import concourse.tile as tile
    from concourse import bacc
    from concourse.alu_op_type import AluOpType

    f32 = mybir.dt.float32
    f32r = mybir.dt.float32r
    Sin = mybir.ActivationFunctionType.Sin
    Abs = mybir.ActivationFunctionType.Abs

    nc = bacc.Bacc("TRN2", target_bir_lowering=False, debug=False,
                   num_devices=NCORES)
    xt_d = nc.dram_tensor("xt", [I, B], f32, kind="ExternalInput").ap()
    w_d = nc.dram_tensor("w", [G_LOC, 2, I, O], f32r, kind="ExternalInput").ap()
    sv_d = nc.dram_tensor("sv", [I, G_LOC], f32, kind="ExternalInput").ap()
    y_d = nc.dram_tensor("yp", [B, O], f32, kind="ExternalOutput").ap()

    with tile.TileContext(nc) as tc:
        with (
            tc.tile_pool(name="inp", bufs=1) as inp,
            tc.tile_pool(name="wpool", bufs=8) as wpool,
            tc.tile_pool(name="trig", bufs=4) as trig,
            tc.tile_pool(name="psum", bufs=1, space="PSUM") as pp,
            tc.tile_pool(name="opool", bufs=4) as opool,
        ):
            sv = inp.tile([I, G_LOC], f32)
            nc.sync.dma_start(sv[:], sv_d)
            xt = inp.tile([I, B], f32)
            bias_ph = inp.tile([I, 1], f32)
            nc.vector.memset(bias_ph[:], float(np.float32(np.pi / 2)))

            # pass-0 slice up front; later slices prefetched mid-pass so the
            # kernel head only waits for 512KB of x + the first coeff tiles
            nc.sync.dma_start(xt[:, 0:BGRP], xt_d[:, 0:BGRP])
            for p in range(NPASS):
                ps = [pp.tile([128, O], f32, tag=f"ps{c}", name=f"ps{c}")
                      for c in range(NCHUNK)]
                xs = xt[:, p * BGRP:(p + 1) * BGRP]
                for g in range(G_LOC):
                    if g == 8 and p + 1 < NPASS:
                        nc.sync.dma_start(
                            xt[:, (p + 1) * BGRP:(p + 2) * BGRP],
                            xt_d[:, (p + 1) * BGRP:(p + 2) * BGRP])
                    wc = wpool.tile([I, O], f32r, tag="wc", name="wc")
                    nc.sync.dma_start(wc[:], w_d[g, 0])
                    ws = wpool.tile([I, O], f32r, tag="ws", name="ws")
                    nc.sync.dma_start(ws[:], w_d[g, 1])

                    a = trig.tile([I, BGRP], f32, tag="a", name="a")
                    n = trig.tile([I, BGRP], f32, tag="n", name="n")
                    f = trig.tile([I, BGRP], f32, tag="f", name="f")
                    fc = trig.tile([I, BGRP], f32, tag="fc", name="fc")
                    sn = trig.tile([I, BGRP], f32r, tag="sn", name="sn")
                    cs = trig.tile([I, BGRP], f32r, tag="cs", name="cs")
                    nc.vector.tensor_scalar(a[:], xs, sv[:, g:g + 1], None,
                                            AluOpType.mult)
                    nc.vector.tensor_scalar(n[:], a[:], MAGIC, MAGIC,
                                            AluOpType.add, AluOpType.subtract)
                    nc.vector.tensor_tensor(f[:], a[:], n[:],
                                            AluOpType.subtract)
                    nc.scalar.activation(sn[:], f[:], Sin, scale=S2PI)
                    if g % 2 == 0:
                        # cos arg on DVE: fc = wrap(f + 0.25) in turns
                        nc.vector.add_range_wrap(fc[:], f[:], 0.25, 0.5, 1.0)
                        nc.scalar.activation(cs[:], fc[:], Sin, scale=S2PI)
                    else:
                        # cos arg on ACT: |f|, then cos = Sin(pi/2 - 2pi|f|)
                        nc.scalar.activation(fc[:], f[:], Abs)
                        nc.scalar.activation(cs[:], fc[:], Sin, scale=-S2PI,
                                             bias=bias_ph[:, 0:1])
                    # sin first: sn is ready ~2us before cs at the kernel head
                    for c in range(NCHUNK):
                        nc.tensor.matmul(ps[c][:],
                                         sn[:, c * 128:(c + 1) * 128],
                                         ws[:], start=(g == 0), stop=False)
                    for c in range(NCHUNK):
                        nc.tensor.matmul(ps[c][:],
                                         cs[:, c * 128:(c + 1) * 128],
                                         wc[:], start=False,
                                         stop=(g == G_LOC - 1))
                for c in range(NCHUNK):
                    o = opool.tile([128, O], f32, tag="o", name="o")
                    nc.vector.tensor_copy(o[:], ps[c][:])
                    nc.sync.dma_start(y_d[p * BGRP + c * 128:
                                          p * BGRP + (c + 1) * 128, :], o[:])

    nc.compile()
    return nc


def _prep(x, fouriercoeffs):
    xt = np.ascontiguousarray(x.T.astype(np.float32, copy=False))  # [I, B]
    wp = np.zeros((GPAD, 2, I, O), dtype=np.float32)
    # fouriercoeffs[d, j, i, g] -> wp[g, d, i, j]
    wp[:G] = fouriercoeffs.transpose(3, 0, 2, 1)
    ks = np.arange(1, GPAD + 1, dtype=np.float64) / (2 * np.pi)
    sva = ks.astype(np.float32)
    sva[G:] = 0.0
    in_maps = []
    for m in range(NCORES):
        sl = slice(m * G_LOC, (m + 1) * G_LOC)
        in_maps.append({
            "xt": xt,
            "w": np.ascontiguousarray(wp[sl]),
            "sv": np.broadcast_to(sva[sl], (I, G_LOC)).copy(),
        })
    return in_maps


def kernel(x, fouriercoeffs):
    global _compiled
    from concourse.bass_utils import run_bass_kernel_spmd

    if _compiled is None:
        _compiled = _build()
    in_maps = _prep(np.asarray(x), np.asarray(fouriercoeffs))
    res = run_bass_kernel_spmd(_compiled, in_maps, core_ids=list(range(NCORES)))
    y = np.zeros((B, O), dtype=np.float64)
    for m in range(NCORES):
        y += res.results[m]["yp"].astype(np.float64)
    return y.astype(np.float32)
